# revision 5
# baseline (speedup 1.0000x reference)
"""Trainium2 Bass kernel for nn_Attention_8744553414813.

Reference (B=4, C=512, H=W=64, HW=4096):
    Q = conv1x1(mean_norm(content), Wq, bq); K = conv1x1(mean_norm(style), Wk, bk)
    V = conv1x1(style, Wv, bv); out = V @ softmax(Q^T K, -1)^T

Sharding: 8 cores = 4 batches x 2 content-pixel halves (data parallel,
weights replicated). Each core computes out^T for its 2048 queries; the
host reassembles.

Algebraic restructuring (host folds parameters, device does the FLOPs): K-projection folded away; V-projection via associativity.

S = Q^T K = xc^T (W'q W'k^T) xs + (W'k b'q)^T xs  (K-side bias is
softmax-invariant and dropped), so with G = W'q W'k^T and beta = W'k b'q
computed on the host, the device needs ONE content projection
Q'' = G^T xc + beta and dots it straight against raw fp16 style pixels.
out^T = (A^T xs^T) Wv eliminates the V projection (U-matmul + 4
transposes + final Wv multiply per tile).

Device per core: Q'' proj (32,768 rows) + 16 q-tiles x [scores 16,384 +
A-transpose 4,096 + U 16,384 + U-transpose 512 + final 2,048 rows]
= 663,552 PE rows total. Flash softmax, fp16 A/U, f32 PSUM throughout.
"""
import numpy as np

import concourse.bacc as bacc
import concourse.bass as bass
import concourse.mybir as mybir
import concourse.tile as tile
from concourse.bass_utils import run_bass_kernel_spmd
from concourse.masks import make_identity

F32 = mybir.dt.float32
F16 = mybir.dt.float16
AF = mybir.ActivationFunctionType
AX = mybir.AxisListType
OP = mybir.AluOpType

B, C, H, W = 4, 512, 64, 64
HW = H * W
QN = HW // 2
CS = C // 128
EPS = 1e-5
KCHUNK = 1024
NKC = HW // KCHUNK
PIX = 512
NCC = QN // PIX
NWARM = 6


def build_nc():
    nc = bacc.Bacc(trn_type="TRN2")
    # style keys channel-major by score-chunk: [ci, kc, sub, kpix]
    xss_d = nc.dram_tensor("xs_s", [128, NKC, CS, KCHUNK], F16, kind="ExternalInput")
    # style pixels pixel-major: [p, kblock, ci] for the U-matmul
    xst_d = nc.dram_tensor("xs_t", [128, HW // 128, C], F16, kind="ExternalInput")
    xc = nc.dram_tensor("xc_p", [128, NCC, CS, PIX], F16, kind="ExternalInput")
    wp = nc.dram_tensor("w_p", [128, 2, CS, C], F16, kind="ExternalInput")  # [G, Wv]
    bp = nc.dram_tensor("b_p", [128, CS + C], F32, kind="ExternalInput")    # beta, bv
    out = nc.dram_tensor("out_t", [QN, C], F32, kind="ExternalOutput")

    with tile.TileContext(nc) as tc:
        with tc.tile_pool(name="sb", bufs=1) as sb, \
             tc.tile_pool(name="cst", bufs=1) as cst, \
             tc.tile_pool(name="xcp", bufs=2) as xcp, \
             tc.tile_pool(name="qc", bufs=2) as qcp, \
             tc.tile_pool(name="ab", bufs=2) as abp, \
             tc.tile_pool(name="atb", bufs=1) as atp, \
             tc.tile_pool(name="ub", bufs=2) as ubp, \
             tc.tile_pool(name="utb", bufs=2) as utp, \
             tc.tile_pool(name="ob", bufs=2) as obp, \
             tc.tile_pool(name="sm", bufs=3) as smp, \
             tc.tile_pool(name="psS", bufs=2, space="PSUM") as psS, \
             tc.tile_pool(name="psT", bufs=2, space="PSUM") as psT, \
             tc.tile_pool(name="psM", bufs=2, space="PSUM") as psM:

            wsb = cst.tile([128, 2, CS, C], F16)
            nc.sync.dma_start(wsb[:, 0], wp[:, 0])     # G gates the first matmul
            bsb = cst.tile([128, CS + C], F32)
            nc.sync.dma_start(bsb[:], bp[:])
            xss = sb.tile([128, NKC, CS, KCHUNK], F16)   # keys, 32 KB/p
            for kc in range(NKC):
                nc.sync.dma_start(xss[:, kc], xss_d[:, kc])
            nc.sync.dma_start(wsb[:, 1], wp[:, 1])
            xsT = sb.tile([128, HW // 128, C], F16)      # U-operand, 32 KB/p
            for c8 in range(4):
                nc.sync.dma_start(xsT[:, c8 * 8:(c8 + 1) * 8, :],
                                  xst_d[:, c8 * 8:(c8 + 1) * 8, :])
            ident = cst.tile([128, 128], F16)
            make_identity(nc, ident)

            for i in range(NWARM):
                wt = psT.tile([128, 1024], F16, tag="tp")
                for j in range(8):
                    nc.tensor.transpose(wt[:, j * 128:(j + 1) * 128], ident[:], ident[:])

            g_r = wsb[:, 0]
            wv_r = wsb[:, 1]
            bq_t = bsb[:, 0:CS]
            bv_t = bsb[:, CS:]

            xqts = [None] * NCC
            qcs = [None] * NCC

            def issue_xq(t):
                xqt = xcp.tile([128, CS, PIX], F16, tag="xcp")
                nc.scalar.dma_start(xqt[:], xc[:, t])
                xqts[t] = xqt

            def emit_qproj(t):
                xqt = xqts[t]
                qc = qcp.tile([128, CS, PIX], F16, tag="qc")
                for co in range(CS):
                    psq = psM.tile([128, PIX], F32, tag="mm512")
                    for ci in range(CS):
                        nc.tensor.matmul(psq[:], g_r[:, ci, co * 128:(co + 1) * 128],
                                         xqt[:, ci, :], start=(ci == 0), stop=(ci == CS - 1))
                    nc.vector.tensor_scalar_add(qc[:, co, :], psq[:], bq_t[:, co:co + 1])
                qcs[t] = qc

            issue_xq(0)
            issue_xq(1)
            emit_qproj(0)

            ctxA = None
            pendB = None

            def flushA_start(p):
                at_p, rd_p, q0_p = p
                att = atp.tile([128, HW // 128, 128], F16, tag="AT", name="att")
                return {"att": att, "at": at_p, "psU": None, "rd": rd_p, "q0": q0_p}

            def flushA_tr(ctx, gs):
                att, at_p = ctx["att"], ctx["at"]
                for g in gs:
                    tp = psT.tile([128, 1024], F16, tag="tp")
                    for i in range(8):
                        kb = g * 8 + i
                        nc.tensor.transpose(tp[:, i * 128:(i + 1) * 128],
                                            at_p[:, kb * 128:(kb + 1) * 128], ident[:])
                    nc.scalar.copy(att[:, g * 8:(g + 1) * 8, :], tp[:])

            def flushA_U(ctx, kb0, kb1):
                att = ctx["att"]
                if ctx["psU"] is None:
                    ctx["psU"] = psM.tile([128, C], F32, tag="mm512", name="psU")
                psU = ctx["psU"]
                for kb in range(kb0, kb1):
                    nc.tensor.matmul(psU[:], att[:, kb, :], xsT[:, kb, :],
                                     start=(kb == 0), stop=(kb == HW // 128 - 1),
                                     skip_group_check=True)

            def flushA_usb(ctx):
                usb = ubp.tile([128, C], F16, tag="U", name="usb")
                nc.scalar.copy(usb[:], ctx["psU"][:])
                return (usb, ctx["rd"], ctx["q0"])

            def flushB(p):
                usb, rd_p, q0_p = p
                ptU = psT.tile([128, C], F16, tag="tp")
                for s in range(CS):
                    nc.tensor.transpose(ptU[:, s * 128:(s + 1) * 128],
                                        usb[:, s * 128:(s + 1) * 128], ident[:])
                uts = utp.tile([128, CS, 128], F16, tag="UT")
                nc.scalar.copy(uts[:], ptU[:])
                av = psM.tile([128, C], F32, tag="mm512")
                for s in range(CS):
                    nc.tensor.matmul(av[:], uts[:, s, :], wv_r[:, s, :],
                                     start=(s == 0), stop=(s == CS - 1))
                ot = obp.tile([128, C], F32, tag="ot")
                nc.vector.scalar_tensor_tensor(ot[:], av[:], rd_p[:], bv_t[:],
                                               OP.mult, OP.add)
                nc.sync.dma_start(out[q0_p:q0_p + 128, :], ot[:])

            for t in range(NCC):
                qc = qcs[t]
                for j in range(PIX // 128):
                    at = abp.tile([128, HW], F16, tag="A")
                    mruns = smp.tile([128, NKC], F32, tag="mruns")
                    negs = smp.tile([128, NKC], F32, tag="negs")
                    dvec = smp.tile([128, NKC], F32, tag="dvec")
                    for kc in range(NKC):
                        sps = psS.tile([128, KCHUNK], F32, tag="s")
                        for kb in range(KCHUNK // PIX):
                            for sub in range(CS):
                                nc.tensor.matmul(sps[:, kb * PIX:(kb + 1) * PIX],
                                                 qc[:, sub, j * 128:(j + 1) * 128],
                                                 xss[:, kc, sub, kb * PIX:(kb + 1) * PIX],
                                                 start=(sub == 0), stop=(sub == CS - 1))
                        if kc == 0:
                            nc.vector.reduce_max(mruns[:, 0:1], sps[:], axis=AX.X)
                        else:
                            mx = smp.tile([128, 1], F32, tag="mx")
                            nc.vector.reduce_max(mx[:], sps[:], axis=AX.X)
                            nc.vector.tensor_tensor(mruns[:, kc:kc + 1], mruns[:, kc - 1:kc],
                                                    mx[:], OP.max)
                        nc.vector.tensor_scalar_mul(negs[:, kc:kc + 1], mruns[:, kc:kc + 1], -1.0)
                        nc.scalar.activation(at[:, kc * KCHUNK:(kc + 1) * KCHUNK], sps[:],
                                             AF.Exp, bias=negs[:, kc:kc + 1], scale=1.0,
                                             accum_out=dvec[:, kc:kc + 1])
                        if ctxA is not None:
                            if kc == 0:
                                flushA_tr(ctxA, (0, 1))
                            elif kc == 1:
                                flushA_tr(ctxA, (2, 3))
                                flushA_U(ctxA, 0, 16)
                            elif kc == 2:
                                flushA_U(ctxA, 16, HW // 128)
                    fac = smp.tile([128, NKC], F32, tag="fac")
                    nc.scalar.activation(fac[:], mruns[:], AF.Exp,
                                         bias=negs[:, NKC - 1:NKC], scale=1.0)
                    dsc = smp.tile([128, NKC], F32, tag="dsc")
                    nc.vector.tensor_tensor(dsc[:], dvec[:], fac[:], OP.mult)
                    dtot = smp.tile([128, 1], F32, tag="dtot")
                    nc.vector.reduce_sum(dtot[:], dsc[:], axis=AX.X)
                    rd = smp.tile([128, 1], F32, tag="rd")
                    nc.vector.reciprocal(rd[:], dtot[:])
                    for kc in range(NKC - 1):
                        nc.vector.tensor_scalar_mul(at[:, kc * KCHUNK:(kc + 1) * KCHUNK],
                                                    at[:, kc * KCHUNK:(kc + 1) * KCHUNK],
                                                    fac[:, kc:kc + 1])
                    if j == 0 and t + 1 < NCC:
                        if t + 2 < NCC:
                            issue_xq(t + 2)
                        emit_qproj(t + 1)
                    nbB = flushA_usb(ctxA) if ctxA is not None else None
                    if pendB is not None:
                        flushB(pendB)
                    pendB = nbB
                    ctxA = flushA_start((at, rd, (t * PIX // 128 + j) * 128))
            flushA_tr(ctxA, (0, 1, 2, 3))
            flushA_U(ctxA, 0, HW // 128)
            nbB = flushA_usb(ctxA)
            if pendB is not None:
                flushB(pendB)
            flushB(nbB)

    nc.compile()
    return nc


_NC = None
_last_in_maps = None


def _get_nc():
    global _NC
    if _NC is None:
        _NC = build_nc()
    return _NC


def _stats(feat):
    x = feat.reshape(C, HW).astype(np.float64)
    mean = x.mean(axis=1)
    var = ((x - mean[:, None]) ** 2).sum(axis=1) / (HW - 1)
    return mean, np.sqrt(var + EPS)


def _pack_w(Wt):
    return np.ascontiguousarray(Wt.reshape(CS, 128, C).transpose(1, 0, 2))


def _pack_xc(x):
    return np.ascontiguousarray(
        x.astype(np.float16).reshape(CS, 128, NCC, PIX).transpose(1, 2, 0, 3))


def kernel(content_feat, style_feat, Wq, bq, Wk, bk, Wv, bv):
    content = np.asarray(content_feat, dtype=np.float32).reshape(B, C, HW)
    style = np.asarray(style_feat, dtype=np.float32).reshape(B, C, HW)
    Wq = np.asarray(Wq, dtype=np.float32)
    Wk = np.asarray(Wk, dtype=np.float32)
    Wv = np.asarray(Wv, dtype=np.float32)
    bq = np.asarray(bq, dtype=np.float32)
    bk = np.asarray(bk, dtype=np.float32)
    bv = np.asarray(bv, dtype=np.float32)

    in_maps = []
    per_batch = {}
    for b in range(B):
        mc, sc = _stats(content[b])
        ms, ss = _stats(style[b])
        Wqp = Wq.T.astype(np.float64) / sc[:, None]      # [cin, cout]
        Wkp = Wk.T.astype(np.float64) / ss[:, None]
        bqp = bq.astype(np.float64) - Wqp.T @ mc
        G = (Wqp @ Wkp.T).astype(np.float16)             # [c, c']
        beta = (Wkp @ bqp).astype(np.float32)            # [c']
        wv_p = Wv.T.astype(np.float16)
        w_p = np.ascontiguousarray(np.stack([_pack_w(G), _pack_w(wv_p)], axis=1))
        b_p = np.empty((128, CS + C), np.float32)
        b_p[:, 0:CS] = beta.reshape(CS, 128).T
        b_p[:, CS:] = bv[None, :]
        xs16 = style[b].astype(np.float16)
        xss = np.ascontiguousarray(                      # [ci, kc, sub, kpix]
            xs16.reshape(CS, 128, NKC, KCHUNK).transpose(1, 2, 0, 3))
        xsT = np.ascontiguousarray(                      # [p, kblock, ci]
            xs16.T.reshape(HW // 128, 128, C).transpose(1, 0, 2))
        per_batch[b] = (w_p, b_p, xss, xsT)

    for core in range(8):
        b = core // 2
        half = core % 2
        w_p, b_p, xss, xsT = per_batch[b]
        xc_half = content[b][:, half * QN:(half + 1) * QN]
        in_maps.append({
            "xs_s": xss,
            "xs_t": xsT,
            "xc_p": _pack_xc(xc_half),
            "w_p": w_p,
            "b_p": b_p,
        })

    global _last_in_maps
    _last_in_maps = in_maps
    nc = _get_nc()
    res = run_bass_kernel_spmd(nc, in_maps, core_ids=list(range(8)))

    outf = np.empty((B, C, HW), dtype=np.float32)
    for core in range(8):
        b = core // 2
        half = core % 2
        ot = np.asarray(res.results[core]["out_t"])
        outf[b, :, half * QN:(half + 1) * QN] = ot.T
    return outf.reshape(B, C, H, W)
